# revision 9
# baseline (speedup 1.0000x reference)
"""Trainium2 Bass kernel for a single attention head (no softmax):

    q = x @ Wq + bq ; k = x @ Wk + bk ; v = x @ Wv + bv     [B,N,H]
    out = ((q @ k^T) * 768**-0.5) @ v                        [B,N,H]

No softmax, so the attention associates:  out = q_scaled @ M  with
M = k^T v [64,64] per batch — the N x N score matrix is never formed.

Sharding: 8 cores = 4 batches x 2 sequence halves. Core c handles batch
c//2, query rows [h*2048, (h+1)*2048) with h = c%2; k/v (hence M) are
computed for the full 4096-row sequence on each core (no collective).

Per-core phase A streams 8 x-tiles (fp16, x^T layout [128,6,512]):
  - k/v are projected DIRECTLY into natural [key,channel] layout:
    per 128-key chunk, 6 accumulating matmuls with lhsT = x^T chunk
    (keys as PE columns) and rhs = [Wk|Wv] packed -> PSUM [128,128]
    (cols 0:64 = k, 64:128 = v). No kT/vT, no PE transposes.
  - per chunk: one fp16 PSUM->SBUF copy, then one accumulating matmul
    M += k_c^T v_c into a persistent PSUM bank.
  - q (own-half tiles): 6 full-row matmuls -> PSUM [64,512]; ACT
    applies bq*scale while converting to fp16 qqT.
  Chains are interleaved in the PE stream so consecutive matmuls hit
  different PSUM banks (hides the systolic-array drain; measured 267
  vs 612 ns/MM at free=512).
Bias generality: k/v biases enter M only via rank-1 terms computable
from sum_j x_j, so the host passes  mcorr = (Wk^T Sx) bv^T +
bk (Wv^T Sx)^T + N bk bv^T  and the device adds it to M.
Phase B: out^T = (M*fp16)^T-contracted qqT in 4 matmuls of 512
queries, each DMA'd out as soon as copied.
"""

import sys

sys.path.insert(0, "/opt/trn_rl_repo")

import contextlib

import numpy as np

import concourse.bass as bass
import concourse.tile as tile
from concourse import bacc, mybir

F32 = mybir.dt.float32
F16 = mybir.dt.float16
AF = mybir.ActivationFunctionType

B, N, E, H = 4, 4096, 768, 64
NCORES = 8
HALF = N // 2  # 2048 query rows per core
NT = 8  # 512-column n-tiles per core (full sequence for k/v)
TS = 512  # n-tile size
EC = E // 128  # 6 contraction chunks
QT = HALF // TS  # 4 query tiles per core (own half)
CPT = TS // 128  # 4 key chunks per tile
SCALE = np.float32(1.0) / np.sqrt(np.float32(E))

_cache = {}
XT_BUFS = 3
NATPS_BUFS = 5
QPS_BUFS = 2
NAT_BUFS = 8


def _build_program(loop_r=1):
    nc = bacc.Bacc(None)
    xp = nc.declare_dram_parameter("xp", [NT, 128, EC, TS], F16, isOutput=False)
    wkv = nc.declare_dram_parameter("wkv", [128, EC, 128], F16, isOutput=False)
    wq = nc.declare_dram_parameter("wq", [128, EC, H], F16, isOutput=False)
    bq = nc.declare_dram_parameter("bq", [H, 1], F32, isOutput=False)
    mcorr = nc.declare_dram_parameter("mcorr", [H, H], F32, isOutput=False)
    out = nc.declare_dram_parameter("out", [H, HALF], F32, isOutput=True)

    with tile.TileContext(nc) as tc:
        with (
            tc.tile_pool(name="const", bufs=1) as const,
            tc.tile_pool(name="big", bufs=1) as big,
            tc.tile_pool(name="xtp", bufs=XT_BUFS) as xtp,
            tc.tile_pool(name="natp", bufs=NAT_BUFS) as natp,
            tc.tile_pool(name="tmpsb", bufs=3) as tmpsb,
        ):
            wkv_t = const.tile([128, EC, 128], F16)
            wq_t = const.tile([128, EC, H], F16)
            bq_t = const.tile([H, 1], F32)
            mcorr_t = const.tile([H, H], F32)
            nc.sync.dma_start(wkv_t[:], wkv[:])
            nc.sync.dma_start(wq_t[:], wq[:])
            nc.sync.dma_start(bq_t[:], bq[:])
            nc.sync.dma_start(mcorr_t[:], mcorr[:])

            qqT = big.tile([H, HALF], F16)  # q^T * scale (own half)
            outT = big.tile([H, HALF], F32)

            loop_cm = (
                tc.For_i(0, loop_r, 1) if loop_r > 1 else contextlib.nullcontext()
            )
            with loop_cm:
                pa_ctx = contextlib.ExitStack()
                mpp = pa_ctx.enter_context(
                    tc.tile_pool(name="mpp", bufs=1, space="PSUM")
                )
                natps = pa_ctx.enter_context(
                    tc.tile_pool(name="natps", bufs=NATPS_BUFS, space="PSUM")
                )
                qps = pa_ctx.enter_context(
                    tc.tile_pool(name="qps", bufs=QPS_BUFS, space="PSUM")
                )
                mps = mpp.tile([H, H], F32)  # M accumulator, lives all of A

                nats = [None] * (NT * CPT)  # SBUF fp16 [128,128] kv chunks

                def emit_m(ci):
                    """One M += k_c^T v_c accumulation matmul."""
                    nat = nats[ci]
                    nc.tensor.matmul(
                        mps[:],
                        nat[:, 0:64],
                        nat[:, 64:128],
                        start=(ci == 0),
                        stop=(ci == NT * CPT - 1),
                        skip_group_check=True,
                    )

                def stage_proj(t, mt):
                    """Project tile t: kv chunks (natural layout) + q.
                    Chains are interleaved (and tile mt's M-accumulation
                    matmuls woven between rounds) so consecutive PE
                    matmuls hit different PSUM banks."""
                    xt = xtp.tile([128, EC, TS], F16, tag="xt")
                    nc.sync.dma_start(xt[:, 0:3, :], xp[t, :, 0:3, :])
                    nc.scalar.dma_start(xt[:, 3:6, :], xp[t, :, 3:6, :])

                    own = t < QT
                    ps_chunks = []
                    qpt = None
                    if own:
                        qpt = qps.tile([H, TS], F32, tag="qp", name=f"qp{t}")
                    for u in range(CPT):
                        ps = natps.tile([128, 128], F32, tag="nps", name=f"nps{t}_{u}")
                        ps_chunks.append(ps)
                    for cc in range(EC):
                        for u in range(CPT):
                            nc.tensor.matmul(
                                ps_chunks[u][:],
                                xt[:, cc, u * 128 : (u + 1) * 128],
                                wkv_t[:, cc, :],
                                start=(cc == 0),
                                stop=(cc == EC - 1),
                            )
                        if own:
                            nc.tensor.matmul(
                                qpt[:],
                                wq_t[:, cc, :],
                                xt[:, cc, :],
                                start=(cc == 0),
                                stop=(cc == EC - 1),
                            )
                        if mt is not None and 1 <= cc <= CPT:
                            emit_m(mt * CPT + cc - 1)
                    for u in range(CPT):
                        nat = natp.tile([128, 128], F16, tag="nat", name=f"nat{t}_{u}")
                        if u % 2 == 0:
                            nc.vector.tensor_copy(nat[:], ps_chunks[u][:])
                        else:
                            nc.scalar.copy(nat[:], ps_chunks[u][:])
                        nats[t * CPT + u] = nat
                    if own:
                        qcols = slice(t * TS, (t + 1) * TS)
                        nc.scalar.activation(
                            qqT[:, qcols], qpt[:], AF.Identity, bias=bq_t[:]
                        )

                for t in range(NT):
                    stage_proj(t, t - 1 if t >= 1 else None)
                for u in range(CPT):
                    emit_m((NT - 1) * CPT + u)

                # ---- finalize M (+ host-side bias correction) ----
                msb = tmpsb.tile([H, H], F16, tag="msb")
                nc.vector.tensor_add(msb[:], mps[:], mcorr_t[:])
                pa_ctx.close()

                pb_ctx = contextlib.ExitStack()
                outp = pb_ctx.enter_context(
                    tc.tile_pool(name="outp", bufs=2, space="PSUM")
                )
                for oc in range(QT):
                    ocols = slice(oc * TS, (oc + 1) * TS)
                    ops = outp.tile([H, TS], F32, tag="ot")
                    nc.tensor.matmul(
                        ops[:], msb[:], qqT[:, ocols], start=True, stop=True
                    )
                    if oc % 2 == 0:
                        nc.vector.tensor_copy(outT[:, ocols], ops[:])
                    else:
                        nc.scalar.copy(outT[:, ocols], ops[:])
                    nc.sync.dma_start(out[:, ocols], outT[:, ocols])
                pb_ctx.close()

    nc.compile()
    return nc


def _prep_inputs(x, Wq, bq, Wk, bk, Wv, bv):
    x = np.asarray(x, dtype=np.float32)
    Wq = np.asarray(Wq, dtype=np.float32)
    Wk = np.asarray(Wk, dtype=np.float32)
    Wv = np.asarray(Wv, dtype=np.float32)
    bq = np.asarray(bq, dtype=np.float32)
    bk = np.asarray(bk, dtype=np.float32)
    bv = np.asarray(bv, dtype=np.float32)

    def prep_w(w):  # [768, M] -> [128, 6, M]
        return np.ascontiguousarray(
            w.reshape(EC, 128, w.shape[1]).transpose(1, 0, 2)
        ).astype(np.float16)

    wkv_p = prep_w(np.concatenate([Wk, Wv], axis=1))
    wq_p = prep_w(Wq * SCALE)
    bq_p = np.ascontiguousarray((bq * SCALE).reshape(H, 1))

    in_maps = []
    for c in range(NCORES):
        b, h = divmod(c, 2)
        own = x[b, h * HALF : (h + 1) * HALF]  # [2048, 768]
        other = x[b, (1 - h) * HALF : (2 - h) * HALF]
        xcat = np.concatenate([own, other], axis=0)  # own-first local order
        xpp = np.ascontiguousarray(
            xcat.reshape(NT, TS, EC, 128).transpose(0, 3, 2, 1)
        ).astype(np.float16)  # [8, 128, 6, 512]
        # k/v biases enter M only through rank-1 terms of sum_j x_j
        sx = x[b].sum(axis=0)  # [768]
        sk = Wk.T @ sx  # [64] = sum_j k0_j
        sv = Wv.T @ sx
        mcorr = (
            np.outer(sk, bv) + np.outer(bk, sv) + float(N) * np.outer(bk, bv)
        ).astype(np.float32)
        in_maps.append(
            {
                "xp": xpp,
                "wkv": wkv_p,
                "wq": wq_p,
                "bq": bq_p,
                "mcorr": mcorr,
            }
        )
    return in_maps


def _get_program(loop_r=1):
    key = ("nc", loop_r)
    if key not in _cache:
        _cache[key] = _build_program(loop_r)
    return _cache[key]


def _run_spmd_once(in_maps):
    from concourse.bass_utils import run_bass_kernel_spmd

    nc = _get_program()
    return run_bass_kernel_spmd(nc, in_maps, list(range(NCORES))).results


def _assemble(results):
    full = np.empty((B, N, H), dtype=np.float32)
    for c in range(NCORES):
        b, h = divmod(c, 2)
        full[b, h * HALF : (h + 1) * HALF, :] = results[c]["out"].T
    return full


def kernel(x, Wq, bq, Wk, bk, Wv, bv):
    in_maps = _prep_inputs(x, Wq, bq, Wk, bk, Wv, bv)
    res = _run_spmd_once(in_maps)
    return _assemble(res)
